# revision 13
# baseline (speedup 1.0000x reference)
"""Trainium2 Bass kernel for the DeepEquilibriumModel (Anderson-accelerated DEQ).

Problem: 12 unrolled iterations of
    f(z) = tanh(z @ W1 + x @ Wx + b1) @ W2 + b2
with Anderson mixing (M=5, beta=1, lam=1e-4) from iteration 5 on.

Sharding: pure data parallelism over the 2048 = B*S rows; 8 cores get 256
rows each (cores 0-3 hold batch 0, cores 4-7 batch 1). Weights replicated.
The Anderson normal equations need global row sums per batch element, done
with a tiny per-group AllReduce ([1,8] floats, groups {0..3} / {4..7}).

v2 structure (vs the straightforward pipeline):
  * u-cache: u_j = f_j @ W1 + xwx is computed for every history slot right
    after f_j is ready, which keeps the PE busy during the dots/AllReduce/
    solve window.  Since the Anderson coefficients sum to 1,
        h_pre_{i+1} = z_{i+1} @ W1 + xwx = sum_k c_k u_{i-k}
    so the next iteration's pre-activation is a cheap linear combination of
    cached u's -- no z @ W1 matmul on the critical path at all.
  * The h-combine runs on DVE+GpSimd (bf16, fp32 coefficients) while the PE
    streams the W2 matmuls chunk by chunk.
  * The z-combine (only needed for the next g = f - z) runs off the critical
    path; z stays fp32 to avoid cancellation in g.
  * bf16 histories for u/g, f32r for f (it feeds the PE), 4x4 solve via
    vectorized Gauss-Jordan on [1,4,5] views.
"""

import numpy as np

from concourse import bacc, bass, mybir, tile
from concourse.bass_utils import run_bass_kernel_spmd

import os as _os

B, S, D, F = 2, 1024, 512, 2048
MAX_ITER, M, LAM = int(_os.environ.get("K_ITERS", "12")), 5, 1e-4
NCORES = 8
RPC = (B * S) // NCORES      # rows per core = 256
KD = D // 128                # 4 k-chunks over D
KF = F // 128                # 16 k-chunks over F
MD = D // 128                # 4 output chunks over D
GRP = 4                      # h-combine groups
GCH = KF // GRP              # f-chunks per group = 4
GW = GCH * RPC               # group width = 1024

FP = mybir.dt.float32
FPR = mybir.dt.float32r
BF = mybir.dt.bfloat16
ALU = mybir.AluOpType
ACT = mybir.ActivationFunctionType

RGROUPS = [[0, 1, 2, 3], [4, 5, 6, 7]]
WT = FPR


def _f32(ap):
    return ap.bitcast(FP)


def _emit(nc: bass.Bass):
    v = nc.vector
    sc = nc.scalar
    gp = nc.gpsimd

    # ---------------- DRAM I/O ----------------
    xT_d = nc.dram_tensor("xT", [D, RPC], WT, kind="ExternalInput")
    W1_d = nc.dram_tensor("W1", [D, F], WT, kind="ExternalInput")
    Wx_d = nc.dram_tensor("Wx", [D, F], WT, kind="ExternalInput")
    W2_d = nc.dram_tensor("W2", [F, D], WT, kind="ExternalInput")
    b1_d = nc.dram_tensor("b1", [F], FP, kind="ExternalInput")
    b2_d = nc.dram_tensor("b2", [D], FP, kind="ExternalInput")
    zout_d = nc.dram_tensor("zT_out", [D, RPC], FP, kind="ExternalOutput")

    with tile.TileContext(nc) as tc:
        with (
            tc.tile_pool(name="const", bufs=1) as cp,
            tc.tile_pool(name="hband", bufs=4) as hp,
            tc.tile_pool(name="comb", bufs=2) as csp,
            tc.tile_pool(name="ps1p", bufs=3, space="PSUM") as pp1,
            tc.tile_pool(name="ps2p", bufs=1, space="PSUM") as pp2,
            tc.tile_pool(name="pssm", bufs=1, space="PSUM") as pps,
            tc.tile_pool(name="dram", bufs=2, space="DRAM") as dp,
        ):
            # ---------------- constants / weights ----------------
            W1p = cp.tile([128, KD * F], WT)          # (k,f) at [:, k*F + f*128]
            W2p = cp.tile([128, KF * D], WT)          # (f,m) at [:, f*D + m*128]
            xwxp = cp.tile([128, KF * RPC], WT)       # f at [:, f*RPC]
            b1t = cp.tile([128, KF], FP)
            b2t = cp.tile([128, MD], FP)
            ones_col = cp.tile([128, 1], FP)
            ones_row = cp.tile([1, 128], FP)
            onesq = cp.tile([128, 128], FP)
            identR = cp.tile([128, 128], WT)
            coeff_sb = cp.tile([128, 8], FP)

            nc.sync.dma_start(b1t[:], b1_d.ap().rearrange("(f p) -> p f", p=128))
            nc.sync.dma_start(b2t[:], b2_d.ap().rearrange("(m p) -> p m", p=128))
            v.memset(ones_col[:], 1.0)
            v.memset(ones_row[:], 1.0)
            v.memset(onesq[:], 1.0)
            gp.affine_select(onesq[:], onesq[:], [[1, 128]], ALU.is_equal, 0.0,
                             base=0, channel_multiplier=-1)
            v.tensor_copy(identR[:], onesq[:])

            # -------- state tiles --------
            fh = [cp.tile([128, KD * RPC], WT, name=f"fh{j}") for j in range(M)]
            gh = [cp.tile([128, KD * RPC], BF, name=f"gh{j}") for j in range(M)]
            z0 = cp.tile([128, KD * RPC], FP)
            z1 = cp.tile([128, KD * RPC], FP)
            dA = cp.tile([128, 1], FP)
            dV = cp.tile([128, 4], FP)
            redp = cp.tile([1, 8], FP)
            red2 = cp.tile([1, 8], FP)
            Pg = [cp.tile([1, 25], FP, name=f"pg{j}") for j in range(2)]
            Au = cp.tile([1, 20], FP)      # augmented [HTH | HTy] as [1,4,5]
            u4 = cp.tile([1, 4], FP)
            st4 = cp.tile([1, 4], FP)
            rcp = cp.tile([1, 1], FP)
            rowp = cp.tile([1, 5], FP)
            t45 = cp.tile([1, 20], FP)
            csum = cp.tile([1, 1], FP)
            coeffs = cp.tile([1, 5], FP)

            v.memset(dA[:], 0.0)
            v.memset(dV[:], 0.0)
            v.memset(redp[:], 0.0)
            v.memset(Pg[0][:], 0.0)
            v.memset(Pg[1][:], 0.0)

            # warm up the collective path (first AllReduce after load pays a
            # large one-time latency).
            n_warm = int(_os.environ.get("K_CC_WARMUP", "2"))
            for w in range(n_warm):
                wcc_in = dp.tile([1, 8], FP, tag="cci", name="wcci")
                wcc_out = dp.tile([1, 8], FP, tag="cco", name="wcco")
                gp.dma_start(wcc_in[:], redp[:])
                gp.collective_compute(
                    "AllReduce", ALU.add, replica_groups=RGROUPS,
                    ins=[wcc_in.opt()], outs=[wcc_out.opt()],
                )

            # ---------------- prolog: xwx = Wx.T @ xT + b1 ----------------
            with tc.tile_pool(name="prolog", bufs=1) as pro:
                xTs = pro.tile([128, KD * RPC], WT)
                Wxp = pro.tile([128, KD * F], WT)
                for k in range(KD):
                    nc.sync.dma_start(xTs[:, k * RPC:(k + 1) * RPC],
                                      xT_d[k * 128:(k + 1) * 128, :])
                for k in range(KD):
                    nc.sync.dma_start(Wxp[:, k * F:(k + 1) * F],
                                      Wx_d[k * 128:(k + 1) * 128, :])
                for f in range(KF):
                    nc.sync.dma_start(W2p[:, f * D:(f + 1) * D],
                                      W2_d[f * 128:(f + 1) * 128, :])
                for k in range(KD):
                    nc.sync.dma_start(W1p[:, k * F:(k + 1) * F],
                                      W1_d[k * 128:(k + 1) * 128, :])
                for f in range(KF):
                    ps1 = pp1.tile([128, RPC], FP, tag="ps1", name="ps1")
                    for k in range(KD):
                        nc.tensor.matmul(
                            ps1[:],
                            Wxp[:, k * F + f * 128: k * F + (f + 1) * 128],
                            xTs[:, k * RPC:(k + 1) * RPC],
                            start=(k == 0), stop=(k == KD - 1),
                        )
                    sc.activation(xwxp[:, f * RPC:(f + 1) * RPC], ps1[:],
                                  ACT.Identity, bias=b1t[:, f:f + 1], scale=1.0)

            # state that reuses the prolog zone (u history, scratch)
            sp2_cm = tc.tile_pool(name="state2", bufs=1)
            sp2 = sp2_cm.__enter__()
            uh = [sp2.tile([128, KF * RPC], BF, name=f"uh{j}") for j in range(M)]
            hpre = [sp2.tile([128, GW], BF, name=f"hpre{j}") for j in range(GRP)]
            junkA = sp2.tile([128, KD * RPC], BF)
            junkV = sp2.tile([128, KD * RPC], BF)

            # ---------------- main loop (fully unrolled) ----------------
            z_cur = None  # fp32 AP of z_i (None for i=0 -> zeros)
            for i in range(MAX_ITER):
                slot = i % M
                last = (i == MAX_ITER - 1)
                f_t, g_t = fh[slot], gh[slot]

                # ---- W2 phase: f_i = tanh(h_pre_i) @ W2 + b2 ----
                ps2 = [pp2.tile([128, RPC], FP, tag=f"ps2_{m}", name=f"ps2_{m}")
                       for m in range(MD)]
                for f in range(KF):
                    if i == 0:
                        src = _f32(xwxp[:, f * RPC:(f + 1) * RPC])
                    elif i <= M:
                        src = uh[(i - 1) % M][:, f * RPC:(f + 1) * RPC]
                    else:
                        src = hpre[f // GCH][:, (f % GCH) * RPC:(f % GCH + 1) * RPC]
                    h = hp.tile([128, RPC], WT, tag="h", name="h")
                    sc.activation(h[:], src, ACT.Tanh)
                    for m in range(MD):
                        nc.tensor.matmul(
                            ps2[m][:],
                            W2p[:, f * D + m * 128: f * D + (m + 1) * 128],
                            h[:],
                            start=(f == 0), stop=(f == KF - 1),
                        )

                # ---- epilogues (ACT) + g (DVE, fused from PSUM) ----
                for m in range(MD):
                    sc.activation(f_t[:, m * RPC:(m + 1) * RPC], ps2[m][:],
                                  ACT.Identity, bias=b2t[:, m:m + 1], scale=1.0)
                for m in range(MD):
                    ms = slice(m * RPC, (m + 1) * RPC)
                    if i == 0:
                        v.tensor_scalar(g_t[:, ms], ps2[m][:], b2t[:, m:m + 1],
                                        None, op0=ALU.add)
                    else:
                        v.scalar_tensor_tensor(g_t[:, ms], ps2[m][:],
                                               b2t[:, m:m + 1], z_cur[:, ms],
                                               op0=ALU.add, op1=ALU.subtract)

                # ---- dots: <g_i, g_{i-j}>, split across ACT/DVE/GpSimd ----
                njd = min(i, M - 1)
                sc.activation(junkA[:], g_t[:], ACT.Square, accum_out=dA[:])
                for j in (1, 2, 3, 4):
                    if j <= njd:
                        v.scalar_tensor_tensor(
                            junkV[:], g_t[:], 1.0, gh[(i - j) % M][:],
                            op0=ALU.bypass, op1=ALU.mult,
                            accum_out=dV[:, j - 1:j])

                # ---- u phase part 1 (PE busy while dots finish) ----
                def u_chunk(f):
                    ps1 = pp1.tile([128, RPC], FP, tag="ps1", name="ps1")
                    nc.tensor.matmul(ps1[:], identR[:],
                                     xwxp[:, f * RPC:(f + 1) * RPC],
                                     start=True, stop=False)
                    for k in range(KD):
                        nc.tensor.matmul(
                            ps1[:],
                            W1p[:, k * F + f * 128: k * F + (f + 1) * 128],
                            f_t[:, k * RPC:(k + 1) * RPC],
                            start=False, stop=(k == KD - 1),
                        )
                    sc.activation(uh[slot][:, f * RPC:(f + 1) * RPC], ps1[:],
                                  ACT.Identity)

                nu_pre = 0 if last else 3
                for f in range(nu_pre):
                    u_chunk(f)

                # ---- partition-reduce the dots, ship to the AllReduce ----
                pball = pps.tile([128, 32], FP, tag="psmall", name="pball")
                psd = pball[0:1, 0:8]
                nc.tensor.matmul(psd[:, 0:1], ones_col[:], dA[:],
                                 start=True, stop=True)
                nc.tensor.matmul(psd[:, 1:5], ones_col[:], dV[:],
                                 start=True, stop=True)
                sc.activation(redp[:], psd, ACT.Copy)

                cc_in = dp.tile([1, 8], FP, tag="cci", name="cci")
                cc_out = dp.tile([1, 8], FP, tag="cco", name="cco")
                nc.sync.dma_start(cc_in[:], redp[:])
                gp.collective_compute(
                    "AllReduce", ALU.add, replica_groups=RGROUPS,
                    ins=[cc_in.opt()], outs=[cc_out.opt()],
                )
                nc.sync.dma_start(red2[:], cc_out[:])

                # ---- u phase part 2 ----
                if not last:
                    for f in range(nu_pre, KF):
                        u_chunk(f)

                # ---- P shift (overlaps the AllReduce) + insert ----
                Pc, Pp = Pg[i % 2], Pg[(i + 1) % 2]
                P3c = Pc[:].rearrange("p (a b) -> p a b", a=5)
                P3p = Pp[:].rearrange("p (a b) -> p a b", a=5)
                v.tensor_copy(P3c[:, 1:5, 1:5], P3p[:, 0:4, 0:4])
                v.tensor_copy(Pc[:, 0:5], red2[:, 0:5])
                v.tensor_copy(Pc[:, 5:25:5], red2[:, 1:5])

                if i < M:
                    z_cur = _f32(f_t[:])
                    continue

                # ---- build augmented [HTH + lam I | HTy] in Au = [1,4,5] ----
                A3 = Au[:].rearrange("p (a b) -> p a b", a=4)
                # A[:, :, 0:4] = Pa0 - Pab
                v.tensor_tensor(A3[:, :, 0:4],
                                P3c[:, 1:5, 0:1].broadcast_to([1, 4, 4]),
                                P3c[:, 1:5, 1:5], op=ALU.subtract)
                # u4[b] = P00 - P0b
                v.scalar_tensor_tensor(u4[:], Pc[:, 1:5], -1.0,
                                       Pc[:, 0:1].broadcast_to([1, 4]),
                                       op0=ALU.mult, op1=ALU.add)
                # HTH = u4 - (Pa0 - Pab)
                v.tensor_tensor(A3[:, :, 0:4],
                                u4[:].rearrange("p (a b) -> p a b", a=1)
                                     .broadcast_to([1, 4, 4]),
                                A3[:, :, 0:4], op=ALU.subtract)
                v.tensor_scalar(st4[:], Au[:, 0:19:6], LAM, None, op0=ALU.add)
                v.tensor_copy(Au[:, 0:19:6], st4[:])
                # HTy[a] = P00 - Pa0
                v.scalar_tensor_tensor(
                    A3[:, :, 4:5],
                    P3c[:, 1:5, 0:1], -1.0,
                    Pc[:, 0:1].rearrange("p (a b) -> p a b", a=1)
                              .broadcast_to([1, 4, 1]),
                    op0=ALU.mult, op1=ALU.add)

                # ---- Gauss-Jordan (no pivoting; HTH is SPD + lam I) ----
                for p in range(4):
                    v.reciprocal(rcp[:], Au[:, p * 6:p * 6 + 1])
                    v.tensor_scalar(rowp[:], Au[:, p * 5:(p + 1) * 5], rcp[:],
                                    None, op0=ALU.mult)
                    v.tensor_tensor(t45[:].rearrange("p (a b) -> p a b", a=4),
                                    A3[:, :, p:p + 1].broadcast_to([1, 4, 5]),
                                    rowp[:].rearrange("p (a b) -> p a b", a=1)
                                           .broadcast_to([1, 4, 5]),
                                    op=ALU.mult)
                    v.tensor_tensor(A3, A3,
                                    t45[:].rearrange("p (a b) -> p a b", a=4),
                                    op=ALU.subtract)
                    v.tensor_copy(Au[:, p * 5:(p + 1) * 5], rowp[:])

                # gamma = Au[:, 4:20:5]; coeffs = [1 - sum(gamma), gamma]
                v.tensor_reduce(csum[:], Au[:, 4:20:5],
                                axis=mybir.AxisListType.X, op=ALU.add)
                v.tensor_scalar(coeffs[:, 0:1], csum[:], -1.0, 1.0,
                                op0=ALU.mult, op1=ALU.add)
                v.tensor_copy(coeffs[:, 1:5], Au[:, 4:20:5])

                # broadcast coeffs to all partitions (PSUM, then SBUF copy)
                psb = pball[:, 8:13]
                nc.tensor.matmul(psb, ones_row[:], coeffs[:],
                                 start=True, stop=True)
                sc.activation(coeff_sb[:, 0:5], psb, ACT.Copy)

                # HAM keep-alive: tiny matmuls keyed on solve-chain tiles so
                # the PE never idles > ~3.4us during the solve window.
                nc.tensor.matmul(pball[0:1, 16:20], ones_col[0:1, :], u4[:],
                                 start=True, stop=True)
                nc.tensor.matmul(pball[0:1, 20:24], ones_col[0:1, :], st4[:],
                                 start=True, stop=True)

                # ---- combines ----
                cs = coeff_sb
                if not last:
                    # h_pre_{i+1}[grp] = sum_k c_k u_{i-k}, groups split
                    # DVE (0,1,2) / GpSimd (3)
                    for grp in range(GRP):
                        eng = v
                        gs = slice(grp * GW, (grp + 1) * GW)
                        tag = "csv"
                        cur = csp.tile([128, GW], BF, tag=tag, name=tag)
                        eng.tensor_scalar(cur[:], uh[slot][:, gs], cs[:, 0:1],
                                          None, op0=ALU.mult)
                        for k in range(1, M):
                            dst_t = hpre[grp] if k == M - 1 else csp.tile(
                                [128, GW], BF, tag=tag, name=tag)
                            eng.scalar_tensor_tensor(
                                dst_t[:], uh[(i - k) % M][:, gs], cs[:, k:k + 1],
                                cur[:], op0=ALU.mult, op1=ALU.add)
                            cur = dst_t

                # z_{i+1} = sum_k c_k f_{i-k} (fp32, for the next g and the
                # final output), halves split DVE / GpSimd
                zn = z0 if (i % 2 == 0) else z1
                HW2 = KD * RPC // 2
                for half in range(2):
                    eng = v
                    hs = slice(half * HW2, (half + 1) * HW2)
                    tag = "zcv"
                    cur = csp.tile([128, HW2], FP, tag=tag, name=tag)
                    eng.tensor_scalar(cur[:], _f32(fh[slot][:])[:, hs],
                                      cs[:, 0:1], None, op0=ALU.mult)
                    for k in range(1, M):
                        dst_t = None
                        if k == M - 1:
                            dst_ap = zn[:, hs]
                        else:
                            dst_t = csp.tile([128, HW2], FP, tag=tag, name=tag)
                            dst_ap = dst_t[:]
                        eng.scalar_tensor_tensor(
                            dst_ap, _f32(fh[(i - k) % M][:])[:, hs],
                            cs[:, k:k + 1], cur[:], op0=ALU.mult, op1=ALU.add)
                        cur = dst_t if dst_t is not None else None
                z_cur = zn[:]

            for k in range(KD):
                nc.sync.dma_start(zout_d[k * 128:(k + 1) * 128, :],
                                  z_cur[:, k * RPC:(k + 1) * RPC])
            sp2_cm.__exit__(None, None, None)

    nc.compile()
    nc.finalize()
    return nc


_NC = None


def _get_nc():
    global _NC
    if _NC is None:
        nc = bacc.Bacc(trn_type="TRN2", debug=False, num_devices=NCORES)
        _NC = _emit(nc)
    return _NC


def kernel(**inputs):
    x = np.ascontiguousarray(np.asarray(inputs["x_input"], dtype=np.float32))
    W1 = np.ascontiguousarray(np.asarray(inputs["W1"], dtype=np.float32))
    Wx = np.ascontiguousarray(np.asarray(inputs["Wx"], dtype=np.float32))
    b1 = np.ascontiguousarray(np.asarray(inputs["b1"], dtype=np.float32))
    W2 = np.ascontiguousarray(np.asarray(inputs["W2"], dtype=np.float32))
    b2 = np.ascontiguousarray(np.asarray(inputs["b2"], dtype=np.float32))

    nc = _get_nc()
    in_maps = []
    for c in range(NCORES):
        b, s0 = c // 4, (c % 4) * RPC
        in_maps.append({
            "xT": np.ascontiguousarray(x[b, s0:s0 + RPC, :].T),
            "W1": W1, "Wx": Wx, "W2": W2, "b1": b1, "b2": b2,
        })
    res = run_bass_kernel_spmd(nc, in_maps, core_ids=list(range(NCORES)))
    out = np.zeros((B, S, D), np.float32)
    for c, om in enumerate(res.results):
        b, s0 = c // 4, (c % 4) * RPC
        out[b, s0:s0 + RPC, :] = om["zT_out"].T
    return out


# revision 24
# speedup vs baseline: 1.1254x; 1.1254x over previous
"""Trainium2 Bass kernel for the DeepEquilibriumModel (Anderson-accelerated DEQ).

Problem: 12 unrolled iterations of
    f(z) = tanh(z @ W1 + x @ Wx + b1) @ W2 + b2
with Anderson mixing (M=5, beta=1, lam=1e-4) from iteration 5 on.

Sharding: pure data parallelism over the 2048 = B*S rows; 8 cores get 256
rows each (cores 0-3 hold batch 0, cores 4-7 batch 1). Weights replicated.
The Anderson normal equations need global row sums per batch element, done
with a tiny per-group AllReduce (groups {0..3} / {4..7}).

v3 structure (tuned to measured engine rates):
  * h-loop: per f-chunk, ps1 = identity@xwx + sum_k W1[k,f]@z[k]; tanh to a
    full h tile.  W2 phase is m-outer (one PSUM bank at a time) so f, g and
    the Gram dot partials trail each m-chunk instead of waiting for all.
  * dots are m-chunked (16 small DVE ops + 4 ACT squares) -> only ~2.5us of
    tail after the last W2 matmul before the AllReduce can launch.
  * Early iterations (0..3) do NO AllReduce: the Gram matrix P is built from
    local partials and reduced ONCE at i=4 (P is linear in the dots), which
    avoids backing up the collective stream.
  * z_{i+1} = sum_k c_k f_{i-k} runs on the PE as 5 scaled-identity matmuls
    per k-chunk (identC_k built by DVE from the broadcast coefficients),
    with ACT copying PSUM->SBUF; ~3us instead of ~8us of DVE stt chain.
  * 4x4 solve via vectorized Gauss-Jordan (SPD + lam*I, no pivoting) on
    [1,4,5] views; builds are fused (~26 tiny DVE ops total).
  * HAM keep-warm: free-running dummy matmuls fill the AllReduce wait and
    solve-keyed dummies tick the PE through the Gauss-Jordan so the clock
    gate never drops the PE to 1.2 GHz mid-iteration.
"""

import numpy as np

from concourse import bacc, bass, mybir, tile
from concourse.bass_utils import run_bass_kernel_spmd

import os as _os

B, S, D, F = 2, 1024, 512, 2048
MAX_ITER, M, LAM = int(_os.environ.get("K_ITERS", "12")), 5, 1e-4
NCORES = 8
RPC = (B * S) // NCORES      # rows per core = 256
KD = D // 128                # 4 k-chunks over D
KF = F // 128                # 16 k-chunks over F
MD = D // 128                # 4 output chunks over D
NDUM = int(_os.environ.get("K_NDUM", "44"))

FP = mybir.dt.float32
FPR = mybir.dt.float32r
BF = mybir.dt.bfloat16
ALU = mybir.AluOpType
ACT = mybir.ActivationFunctionType

RGROUPS = [[0, 1, 2, 3], [4, 5, 6, 7]]
WT = FPR


def _f32(ap):
    return ap.bitcast(FP)


def _emit(nc: bass.Bass):
    v = nc.vector
    sc = nc.scalar
    gp = nc.gpsimd

    # ---------------- DRAM I/O ----------------
    xT_d = nc.dram_tensor("xT", [D, RPC], WT, kind="ExternalInput")
    W1_d = nc.dram_tensor("W1", [D, F], WT, kind="ExternalInput")
    Wx_d = nc.dram_tensor("Wx", [D, F], WT, kind="ExternalInput")
    W2_d = nc.dram_tensor("W2", [F, D], WT, kind="ExternalInput")
    b1_d = nc.dram_tensor("b1", [F], FP, kind="ExternalInput")
    b2_d = nc.dram_tensor("b2", [D], FP, kind="ExternalInput")
    zout_d = nc.dram_tensor("zT_out", [D, RPC], FP, kind="ExternalOutput")

    with tile.TileContext(nc) as tc:
        with (
            tc.tile_pool(name="const", bufs=1) as cp,
            tc.tile_pool(name="ps1p", bufs=2, space="PSUM") as pp1,
            tc.tile_pool(name="ps2p", bufs=2, space="PSUM") as pp2,
            tc.tile_pool(name="pszp", bufs=2, space="PSUM") as ppz,
            tc.tile_pool(name="pssm", bufs=1, space="PSUM") as pps,
            tc.tile_pool(name="dram", bufs=2, space="DRAM") as dp,
        ):
            # ---------------- constants / weights ----------------
            W1p = cp.tile([128, KD * F], WT)          # (k,f) at [:, k*F + f*128]
            W2p = cp.tile([128, KF * D], WT)          # (f,m) at [:, f*D + m*128]
            xwxp = cp.tile([128, KF * RPC], WT)       # f at [:, f*RPC]
            b1t = cp.tile([128, KF], FP)
            b2t = cp.tile([128, MD], FP)
            ones_col = cp.tile([128, 1], FP)
            ones_row = cp.tile([1, 128], FP)
            onesq = cp.tile([128, 128], FP)
            identR = cp.tile([128, 128], WT)
            identC = [cp.tile([128, 128], WT, name=f"idc{j}") for j in range(M)]
            dummv = cp.tile([128, 512], WT)

            nc.sync.dma_start(b1t[:], b1_d.ap().rearrange("(f p) -> p f", p=128))
            nc.sync.dma_start(b2t[:], b2_d.ap().rearrange("(m p) -> p m", p=128))
            v.memset(ones_col[:], 1.0)
            v.memset(ones_row[:], 1.0)
            v.memset(onesq[:], 1.0)
            v.memset(_f32(dummv[:]), 0.0)
            gp.affine_select(onesq[:], onesq[:], [[1, 128]], ALU.is_equal, 0.0,
                             base=0, channel_multiplier=-1)
            v.tensor_copy(identR[:], onesq[:])

            # -------- state tiles --------
            fh = [cp.tile([128, KD * RPC], WT, name=f"fh{j}") for j in range(M)]
            gh = [cp.tile([128, KD * RPC], BF, name=f"gh{j}") for j in range(M)]
            z0 = cp.tile([128, KD * RPC], WT)
            z1 = cp.tile([128, KD * RPC], WT)
            hfull = cp.tile([128, KF * RPC], WT)
            junkV = cp.tile([128, RPC], BF)
            junkA = cp.tile([128, RPC], BF)
            dAm = cp.tile([128, 4], FP)               # <g,g> partials by m
            dVm = cp.tile([128, 16], FP)              # (j,m) partials, j-major
            redp = cp.tile([1, 24], FP)
            red2 = cp.tile([1, 24], FP)
            r5 = cp.tile([1, 5], FP)
            Pg = [cp.tile([1, 25], FP, name=f"pg{j}") for j in range(2)]
            Au = cp.tile([1, 20], FP)      # augmented [HTH | HTy] as [1,4,5]
            u4 = cp.tile([1, 4], FP)
            st4 = cp.tile([1, 4], FP)
            rcp = cp.tile([1, 1], FP)
            rowp = cp.tile([1, 5], FP)
            t45 = cp.tile([1, 20], FP)
            csum = cp.tile([1, 1], FP)
            coeffs = cp.tile([1, 5], FP)

            v.memset(dAm[:], 0.0)
            v.memset(dVm[:], 0.0)
            v.memset(redp[:], 0.0)
            v.memset(Pg[0][:], 0.0)
            v.memset(Pg[1][:], 0.0)

            # warm up the collective path (first AllReduce after load pays a
            # large one-time latency).
            n_warm = int(_os.environ.get("K_CC_WARMUP", "2"))
            for w in range(n_warm):
                wcc_in = dp.tile([1, 24], FP, tag="cci", name="wcci")
                wcc_out = dp.tile([1, 24], FP, tag="cco", name="wcco")
                gp.dma_start(wcc_in[:], redp[:])
                gp.collective_compute(
                    "AllReduce", ALU.add, replica_groups=RGROUPS,
                    ins=[wcc_in.opt()], outs=[wcc_out.opt()],
                )

            # ---------------- prolog: xwx = Wx.T @ xT + b1 ----------------
            with tc.tile_pool(name="prolog", bufs=1) as pro:
                xTs = pro.tile([128, KD * RPC], WT)
                Wxp = pro.tile([128, KD * F], WT)
                for k in range(KD):
                    nc.sync.dma_start(xTs[:, k * RPC:(k + 1) * RPC],
                                      xT_d[k * 128:(k + 1) * 128, :])
                for k in range(KD):
                    nc.sync.dma_start(Wxp[:, k * F:(k + 1) * F],
                                      Wx_d[k * 128:(k + 1) * 128, :])
                for f in range(KF):
                    nc.sync.dma_start(W2p[:, f * D:(f + 1) * D],
                                      W2_d[f * 128:(f + 1) * 128, :])
                for k in range(KD):
                    nc.sync.dma_start(W1p[:, k * F:(k + 1) * F],
                                      W1_d[k * 128:(k + 1) * 128, :])
                for f in range(KF):
                    ps1 = pp1.tile([128, RPC], FP, tag="ps1", name="ps1")
                    for k in range(KD):
                        nc.tensor.matmul(
                            ps1[:],
                            Wxp[:, k * F + f * 128: k * F + (f + 1) * 128],
                            xTs[:, k * RPC:(k + 1) * RPC],
                            start=(k == 0), stop=(k == KD - 1),
                        )
                    sc.activation(xwxp[:, f * RPC:(f + 1) * RPC], ps1[:],
                                  ACT.Identity, bias=b1t[:, f:f + 1], scale=1.0)

            # ---------------- main loop (fully unrolled) ----------------
            z_mm = None   # f32r AP of z_i for matmuls (None for i=0 -> zeros)
            for i in range(MAX_ITER):
                slot = i % M
                last = (i == MAX_ITER - 1)
                f_t, g_t = fh[slot], gh[slot]
                z_f32 = _f32(z_mm) if z_mm is not None else None

                # ---- h phase: hfull = tanh(z @ W1 + xwx) ----
                for f in range(KF):
                    fs = slice(f * RPC, (f + 1) * RPC)
                    if i == 0:
                        sc.activation(hfull[:, fs], _f32(xwxp[:, fs]), ACT.Tanh)
                        continue
                    ps1 = pp1.tile([128, RPC], FP, tag="ps1", name="ps1")
                    nc.tensor.matmul(ps1[:], identR[:], xwxp[:, fs],
                                     start=True, stop=False)
                    for k in range(KD):
                        nc.tensor.matmul(
                            ps1[:],
                            W1p[:, k * F + f * 128: k * F + (f + 1) * 128],
                            z_mm[:, k * RPC:(k + 1) * RPC],
                            start=False, stop=(k == KD - 1),
                        )
                    sc.activation(hfull[:, fs], ps1[:], ACT.Tanh)

                # ---- W2 phase, m-outer; f/g/dot partials trail each m ----
                njd = min(i, M - 1)
                for m in range(MD):
                    ms = slice(m * RPC, (m + 1) * RPC)
                    ps2 = pp2.tile([128, RPC], FP, tag="ps2", name="ps2")
                    for f in range(KF):
                        nc.tensor.matmul(
                            ps2[:],
                            W2p[:, f * D + m * 128: f * D + (m + 1) * 128],
                            hfull[:, f * RPC:(f + 1) * RPC],
                            start=(f == 0), stop=(f == KF - 1),
                        )
                    sc.activation(f_t[:, ms], ps2[:],
                                  ACT.Identity, bias=b2t[:, m:m + 1], scale=1.0)
                    if i == 0:
                        v.tensor_scalar(g_t[:, ms], ps2[:], b2t[:, m:m + 1],
                                        None, op0=ALU.add)
                    else:
                        v.scalar_tensor_tensor(g_t[:, ms], ps2[:],
                                               b2t[:, m:m + 1], z_f32[:, ms],
                                               op0=ALU.add, op1=ALU.subtract)
                    sc.activation(junkA[:], g_t[:, ms], ACT.Square,
                                  accum_out=dAm[:, m:m + 1])
                    for j in range(1, njd + 1):
                        v.scalar_tensor_tensor(
                            junkV[:], g_t[:, ms], 1.0, gh[(i - j) % M][:, ms],
                            op0=ALU.bypass, op1=ALU.mult,
                            accum_out=dVm[:, (j - 1) * 4 + m:(j - 1) * 4 + m + 1])

                # ---- partition-reduce dot partials ----
                pball = pps.tile([128, 32], FP, tag="psmall", name="pball")
                psd = pball[0:1, 0:20]
                nc.tensor.matmul(psd[:, 0:4], ones_col[:], dAm[:],
                                 start=True, stop=True)
                nc.tensor.matmul(psd[:, 4:20], ones_col[:], dVm[:],
                                 start=True, stop=True)
                sc.activation(redp[:, 0:20], psd, ACT.Copy)

                do_ar = i >= M
                if do_ar:
                    cc_in = dp.tile([1, 24], FP, tag="cci", name="cci")
                    cc_out = dp.tile([1, 24], FP, tag="cco", name="cco")
                    nc.sync.dma_start(cc_in[:], redp[:])
                    gp.collective_compute(
                        "AllReduce", ALU.add, replica_groups=RGROUPS,
                        ins=[cc_in.opt()], outs=[cc_out.opt()],
                    )

                # HAM keep-warm: free-running junk matmuls fill the AllReduce
                # wait (each ~0.2us, no dependencies, retire instantly).
                if do_ar and not last:
                    pdum = pps.tile([1, 512], FP, tag="dum", name="pdum")
                    for _ in range(NDUM):
                        nc.tensor.matmul(pdum[:], dummv[0:1, 0:1],
                                         dummv[0:1, 0:512],
                                         start=True, stop=True)

                if do_ar:
                    nc.sync.dma_start(red2[:], cc_out[:])
                    # r5[j] = sum over the 4 m-partials of dot j
                    v.tensor_reduce(r5[:],
                                    red2[:, 0:20].rearrange(
                                        "p (j m) -> p j m", j=5),
                                    axis=mybir.AxisListType.X, op=ALU.add)
                else:
                    v.tensor_reduce(r5[:],
                                    redp[:, 0:20].rearrange(
                                        "p (j m) -> p j m", j=5),
                                    axis=mybir.AxisListType.X, op=ALU.add)

                # ---- P shift + insert (r5 cols: [<g,g>, j1..j4]) ----
                Pc, Pp = Pg[i % 2], Pg[(i + 1) % 2]
                P3c = Pc[:].rearrange("p (a b) -> p a b", a=5)
                P3p = Pp[:].rearrange("p (a b) -> p a b", a=5)
                v.tensor_copy(P3c[:, 1:5, 1:5], P3p[:, 0:4, 0:4])
                v.tensor_copy(Pc[:, 0:5], r5[:, 0:5])
                v.tensor_copy(Pc[:, 5:25:5], r5[:, 1:5])

                if i < M:
                    if i == M - 1:
                        # one AllReduce turns the local Gram into global
                        ccp_in = dp.tile([1, 25], FP, tag="cpi", name="cpi")
                        ccp_out = dp.tile([1, 25], FP, tag="cpo", name="cpo")
                        nc.sync.dma_start(ccp_in[:], Pc[:])
                        gp.collective_compute(
                            "AllReduce", ALU.add, replica_groups=RGROUPS,
                            ins=[ccp_in.opt()], outs=[ccp_out.opt()],
                        )
                        nc.sync.dma_start(Pc[:], ccp_out[:])
                    z_mm = f_t[:]
                    continue

                # ---- build augmented [HTH + lam I | HTy] in Au = [1,4,5] ----
                A3 = Au[:].rearrange("p (a b) -> p a b", a=4)
                v.tensor_tensor(A3[:, :, 0:4],
                                P3c[:, 1:5, 0:1].broadcast_to([1, 4, 4]),
                                P3c[:, 1:5, 1:5], op=ALU.subtract)
                v.scalar_tensor_tensor(u4[:], Pc[:, 1:5], -1.0,
                                       Pc[:, 0:1].broadcast_to([1, 4]),
                                       op0=ALU.mult, op1=ALU.add)
                v.tensor_tensor(A3[:, :, 0:4],
                                u4[:].rearrange("p (a b) -> p a b", a=1)
                                     .broadcast_to([1, 4, 4]),
                                A3[:, :, 0:4], op=ALU.subtract)
                v.tensor_scalar(st4[:], Au[:, 0:19:6], LAM, None, op0=ALU.add)
                v.tensor_copy(Au[:, 0:19:6], st4[:])
                v.scalar_tensor_tensor(
                    A3[:, :, 4:5],
                    P3c[:, 1:5, 0:1], -1.0,
                    Pc[:, 0:1].rearrange("p (a b) -> p a b", a=1)
                              .broadcast_to([1, 4, 1]),
                    op0=ALU.mult, op1=ALU.add)

                # ---- Gauss-Jordan (no pivoting; HTH is SPD + lam I) ----
                # a dummy PE matmul after each pivot keeps the clock hot
                for p in range(4):
                    v.reciprocal(rcp[:], Au[:, p * 6:p * 6 + 1])
                    v.tensor_scalar(rowp[:], Au[:, p * 5:(p + 1) * 5], rcp[:],
                                    None, op0=ALU.mult)
                    v.tensor_tensor(t45[:].rearrange("p (a b) -> p a b", a=4),
                                    A3[:, :, p:p + 1].broadcast_to([1, 4, 5]),
                                    rowp[:].rearrange("p (a b) -> p a b", a=1)
                                           .broadcast_to([1, 4, 5]),
                                    op=ALU.mult)
                    v.tensor_tensor(A3, A3,
                                    t45[:].rearrange("p (a b) -> p a b", a=4),
                                    op=ALU.subtract)
                    v.tensor_copy(Au[:, p * 5:(p + 1) * 5], rowp[:])
                    if not last:
                        nc.tensor.matmul(pball[0:1, 28:32], ones_col[0:1, :],
                                         rowp[:, 0:4], start=True, stop=True)

                # gamma = Au[:, 4:20:5]; coeffs = [1 - sum(gamma), gamma]
                v.tensor_reduce(csum[:], Au[:, 4:20:5],
                                axis=mybir.AxisListType.X, op=ALU.add)
                v.tensor_scalar(coeffs[:, 0:1], csum[:], -1.0, 1.0,
                                op0=ALU.mult, op1=ALU.add)
                v.tensor_copy(coeffs[:, 1:5], Au[:, 4:20:5])

                # broadcast coeffs to all partitions, build scaled identities
                psb = pball[:, 20:25]
                nc.tensor.matmul(psb, ones_row[:], coeffs[:],
                                 start=True, stop=True)
                for j in range(M):
                    v.tensor_scalar(identC[j][:], _f32(identR[:]),
                                    psb[:, j:j + 1], None, op0=ALU.mult)

                # ---- z_{i+1} = sum_k c_k f_{i-k} on the PE ----
                zn = z0 if (i % 2 == 0) else z1
                for kc in range(KD):
                    ks = slice(kc * RPC, (kc + 1) * RPC)
                    psz = ppz.tile([128, RPC], FP, tag="psz", name="psz")
                    for j in range(M):
                        nc.tensor.matmul(psz[:], identC[j][:],
                                         fh[(i - j) % M][:, ks],
                                         start=(j == 0), stop=(j == M - 1))
                    sc.activation(zn[:, ks], psz[:], ACT.Identity)
                z_mm = zn[:]

            for k in range(KD):
                nc.sync.dma_start(zout_d[k * 128:(k + 1) * 128, :],
                                  _f32(z_mm)[:, k * RPC:(k + 1) * RPC])

    nc.compile()
    nc.finalize()
    return nc


_NC = None


def _get_nc():
    global _NC
    if _NC is None:
        nc = bacc.Bacc(trn_type="TRN2", debug=False, num_devices=NCORES)
        _NC = _emit(nc)
    return _NC


def kernel(**inputs):
    x = np.ascontiguousarray(np.asarray(inputs["x_input"], dtype=np.float32))
    W1 = np.ascontiguousarray(np.asarray(inputs["W1"], dtype=np.float32))
    Wx = np.ascontiguousarray(np.asarray(inputs["Wx"], dtype=np.float32))
    b1 = np.ascontiguousarray(np.asarray(inputs["b1"], dtype=np.float32))
    W2 = np.ascontiguousarray(np.asarray(inputs["W2"], dtype=np.float32))
    b2 = np.ascontiguousarray(np.asarray(inputs["b2"], dtype=np.float32))

    nc = _get_nc()
    in_maps = []
    for c in range(NCORES):
        b, s0 = c // 4, (c % 4) * RPC
        in_maps.append({
            "xT": np.ascontiguousarray(x[b, s0:s0 + RPC, :].T),
            "W1": W1, "Wx": Wx, "W2": W2, "b1": b1, "b2": b2,
        })
    res = run_bass_kernel_spmd(nc, in_maps, core_ids=list(range(NCORES)))
    out = np.zeros((B, S, D), np.float32)
    for c, om in enumerate(res.results):
        b, s0 = c // 4, (c % 4) * RPC
        out[b, s0:s0 + RPC, :] = om["zT_out"].T
    return out


# revision 37
# speedup vs baseline: 1.1911x; 1.0583x over previous
"""Trainium2 Bass kernel for the DeepEquilibriumModel (Anderson-accelerated DEQ).

Problem: 12 unrolled iterations of
    f(z) = tanh(z @ W1 + x @ Wx + b1) @ W2 + b2
with Anderson mixing (M=5, beta=1, lam=1e-4) from iteration 5 on.

Sharding: pure data parallelism over the 2048 = B*S rows; 8 cores get 256
rows each (cores 0-3 hold batch 0, cores 4-7 batch 1). Weights replicated.
The Anderson normal equations need global row sums per batch element, done
with a tiny per-group AllReduce (groups {0..3} / {4..7}).

v3 structure (tuned to measured engine rates):
  * h-loop: per f-chunk, ps1 = identity@xwx + sum_k W1[k,f]@z[k]; tanh to a
    full h tile.  W2 phase is m-outer (one PSUM bank at a time) so f, g and
    the Gram dot partials trail each m-chunk instead of waiting for all.
  * dots are m-chunked (16 small DVE ops + 4 ACT squares) -> only ~2.5us of
    tail after the last W2 matmul before the AllReduce can launch.
  * Early iterations (0..3) do NO AllReduce: the Gram matrix P is built from
    local partials and reduced ONCE at i=4 (P is linear in the dots), which
    avoids backing up the collective stream.
  * z_{i+1} = sum_k c_k f_{i-k} runs on the PE as 5 scaled-identity matmuls
    per k-chunk (identC_k built by DVE from the broadcast coefficients),
    with ACT copying PSUM->SBUF; ~3us instead of ~8us of DVE stt chain.
  * 4x4 solve via vectorized Gauss-Jordan (SPD + lam*I, no pivoting) on
    [1,4,5] views; builds are fused (~26 tiny DVE ops total).
  * HAM keep-warm: free-running dummy matmuls fill the AllReduce wait and
    solve-keyed dummies tick the PE through the Gauss-Jordan so the clock
    gate never drops the PE to 1.2 GHz mid-iteration.
"""

import numpy as np

from concourse import bacc, bass, mybir, tile
from concourse.bass_utils import run_bass_kernel_spmd

import os as _os

B, S, D, F = 2, 1024, 512, 2048
MAX_ITER, M, LAM = int(_os.environ.get("K_ITERS", "12")), 5, 1e-4
NCORES = 8
RPC = (B * S) // NCORES      # rows per core = 256
KD = D // 128                # 4 k-chunks over D
KF = F // 128                # 16 k-chunks over F
MD = D // 128                # 4 output chunks over D
NDUM = int(_os.environ.get("K_NDUM", "30"))

FP = mybir.dt.float32
FPR = mybir.dt.float32r
BF = mybir.dt.bfloat16
ALU = mybir.AluOpType
ACT = mybir.ActivationFunctionType

RGROUPS = [[0, 1, 2, 3], [4, 5, 6, 7]]
WT = FPR


def _f32(ap):
    return ap.bitcast(FP)


def _emit(nc: bass.Bass):
    v = nc.vector
    sc = nc.scalar
    gp = nc.gpsimd

    # ---------------- DRAM I/O ----------------
    xT_d = nc.dram_tensor("xT", [D, RPC], WT, kind="ExternalInput")
    W1_d = nc.dram_tensor("W1", [D, F], WT, kind="ExternalInput")
    Wx_d = nc.dram_tensor("Wx", [D, F], WT, kind="ExternalInput")
    W2_d = nc.dram_tensor("W2", [F, D], WT, kind="ExternalInput")
    b1_d = nc.dram_tensor("b1", [F], FP, kind="ExternalInput")
    b2_d = nc.dram_tensor("b2", [D], FP, kind="ExternalInput")
    zout_d = nc.dram_tensor("zT_out", [D, RPC], FP, kind="ExternalOutput")

    with tile.TileContext(nc) as tc:
        with (
            tc.tile_pool(name="const", bufs=1) as cp,
            tc.tile_pool(name="ps1p", bufs=2, space="PSUM") as pp1,
            tc.tile_pool(name="ps2p", bufs=2, space="PSUM") as pp2,
            tc.tile_pool(name="pszp", bufs=2, space="PSUM") as ppz,
            tc.tile_pool(name="pssm", bufs=1, space="PSUM") as pps,
            tc.tile_pool(name="dram", bufs=2, space="DRAM") as dp,
        ):
            # ---------------- constants / weights ----------------
            W1p = cp.tile([128, KD * F], WT)          # (k,f) at [:, k*F + f*128]
            W2p = cp.tile([128, KF * D], WT)          # (f,m) at [:, f*D + m*128]
            xwxp = cp.tile([128, KF * RPC], WT)       # f at [:, f*RPC]
            b1t = cp.tile([128, KF], FP)
            b2t = cp.tile([128, MD], FP)
            ones_col = cp.tile([128, 1], FP)
            ones_row = cp.tile([1, 128], FP)
            onesq = cp.tile([128, 128], FP)
            identR = cp.tile([128, 128], WT)
            identC = [cp.tile([128, 128], WT, name=f"idc{j}") for j in range(M)]

            nc.sync.dma_start(b1t[:], b1_d.ap().rearrange("(f p) -> p f", p=128))
            nc.sync.dma_start(b2t[:], b2_d.ap().rearrange("(m p) -> p m", p=128))
            v.memset(ones_col[:], 1.0)
            v.memset(ones_row[:], 1.0)
            v.memset(onesq[:], 1.0)
            gp.affine_select(onesq[:], onesq[:], [[1, 128]], ALU.is_equal, 0.0,
                             base=0, channel_multiplier=-1)
            v.tensor_copy(identR[:], onesq[:])

            # -------- state tiles --------
            fh = [cp.tile([128, KD * RPC], WT, name=f"fh{j}") for j in range(M)]
            gh = [cp.tile([128, KD * RPC], BF, name=f"gh{j}") for j in range(M)]
            z0 = cp.tile([128, KD * RPC], WT)
            z1 = cp.tile([128, KD * RPC], WT)
            hfull = cp.tile([128, KF * RPC], WT)
            junkV = cp.tile([128, RPC], BF)
            junkA = cp.tile([128, RPC], BF)
            dAm = cp.tile([128, 4], FP)               # <g,g> partials by m
            dVm = cp.tile([128, 16], FP)              # (j,m) partials, j-major
            redp = cp.tile([1, 24], FP)
            red2 = cp.tile([1, 24], FP)
            r5 = cp.tile([1, 5], FP)
            Pg = [cp.tile([1, 25], FP, name=f"pg{j}") for j in range(2)]
            Au = cp.tile([1, 20], FP)      # augmented [HTH | HTy] as [1,4,5]
            u4 = cp.tile([1, 4], FP)
            st4 = cp.tile([1, 4], FP)
            rcp = cp.tile([1, 1], FP)
            rowp = cp.tile([1, 5], FP)
            t45 = cp.tile([1, 20], FP)
            csum = cp.tile([1, 1], FP)
            coeffs = cp.tile([1, 5], FP)
            dumout = cp.tile([1, 4], FP)
            pacev = cp.tile([1, 4], WT)

            v.memset(dAm[:], 0.0)
            v.memset(dVm[:], 0.0)
            v.memset(redp[:], 0.0)
            v.memset(Pg[0][:], 0.0)
            v.memset(Pg[1][:], 0.0)

            # warm up the collective path (first AllReduce after load pays a
            # large one-time latency).
            n_warm = int(_os.environ.get("K_CC_WARMUP", "2"))

            def warm_ar():
                wcc_in = dp.tile([1, 49], FP, tag="cci", name="wcci")
                wcc_out = dp.tile([1, 49], FP, tag="cco", name="wcco")
                gp.dma_start(wcc_in[0:1, 0:24], redp[:])
                gp.collective_compute(
                    "AllReduce", ALU.add, replica_groups=RGROUPS,
                    ins=[wcc_in.opt()], outs=[wcc_out.opt()],
                )

            for w in range(n_warm):
                warm_ar()

            # ---------------- prolog: xwx = Wx.T @ xT + b1 ----------------
            with tc.tile_pool(name="prolog", bufs=1) as pro:
                xTs = pro.tile([128, KD * RPC], WT)
                Wxp = pro.tile([128, KD * F], WT)
                # two hardware DMA queues in parallel: Wx on the scalar
                # engine's queue, everything else on sync
                for k in range(KD):
                    sc.dma_start(Wxp[:, k * F:(k + 1) * F],
                                 Wx_d[k * 128:(k + 1) * 128, :])
                for k in range(KD):
                    nc.sync.dma_start(xTs[:, k * RPC:(k + 1) * RPC],
                                      xT_d[k * 128:(k + 1) * 128, :])
                for f in range(KF):
                    nc.sync.dma_start(W2p[:, f * D:(f + 1) * D],
                                      W2_d[f * 128:(f + 1) * 128, :])
                for k in range(KD):
                    nc.sync.dma_start(W1p[:, k * F:(k + 1) * F],
                                      W1_d[k * 128:(k + 1) * 128, :])
                for f in range(KF):
                    ps1 = pp1.tile([128, RPC], FP, tag="ps1", name="ps1")
                    for k in range(KD):
                        nc.tensor.matmul(
                            ps1[:],
                            Wxp[:, k * F + f * 128: k * F + (f + 1) * 128],
                            xTs[:, k * RPC:(k + 1) * RPC],
                            start=(k == 0), stop=(k == KD - 1),
                        )
                    sc.activation(xwxp[:, f * RPC:(f + 1) * RPC], ps1[:],
                                  ACT.Identity, bias=b1t[:, f:f + 1], scale=1.0)

            # ---------------- main loop (fully unrolled) ----------------
            z_mm = None   # f32r AP of z_i for matmuls (None for i=0 -> zeros)
            for i in range(MAX_ITER):
                slot = i % M
                last = (i == MAX_ITER - 1)
                f_t, g_t = fh[slot], gh[slot]
                z_f32 = _f32(z_mm) if z_mm is not None else None

                # ---- h phase: hfull = tanh(z @ W1 + xwx) ----
                for f in range(KF):
                    fs = slice(f * RPC, (f + 1) * RPC)
                    if i == 0:
                        sc.activation(hfull[:, fs], _f32(xwxp[:, fs]), ACT.Tanh)
                        continue
                    ps1 = pp1.tile([128, RPC], FP, tag="ps1", name="ps1")
                    nc.tensor.matmul(ps1[:], identR[:], xwxp[:, fs],
                                     start=True, stop=False)
                    for k in range(KD):
                        nc.tensor.matmul(
                            ps1[:],
                            W1p[:, k * F + f * 128: k * F + (f + 1) * 128],
                            z_mm[:, k * RPC:(k + 1) * RPC],
                            start=False, stop=(k == KD - 1),
                        )
                    sc.activation(hfull[:, fs], ps1[:], ACT.Tanh)

                # ---- W2 phase, m-outer; f/g/dot partials trail each m ----
                njd = min(i, M - 1)
                for m in range(MD):
                    ms = slice(m * RPC, (m + 1) * RPC)
                    ps2 = pp2.tile([128, RPC], FP, tag="ps2", name="ps2")
                    for f in range(KF):
                        nc.tensor.matmul(
                            ps2[:],
                            W2p[:, f * D + m * 128: f * D + (m + 1) * 128],
                            hfull[:, f * RPC:(f + 1) * RPC],
                            start=(f == 0), stop=(f == KF - 1),
                        )
                    sc.activation(f_t[:, ms], ps2[:],
                                  ACT.Identity, bias=b2t[:, m:m + 1], scale=1.0)
                    if i == 0:
                        v.tensor_scalar(g_t[:, ms], ps2[:], b2t[:, m:m + 1],
                                        None, op0=ALU.add)
                    else:
                        v.scalar_tensor_tensor(g_t[:, ms], ps2[:],
                                               b2t[:, m:m + 1], z_f32[:, ms],
                                               op0=ALU.add, op1=ALU.subtract)
                    sc.activation(junkA[:], g_t[:, ms], ACT.Square,
                                  accum_out=dAm[:, m:m + 1])
                    for j in range(1, njd + 1):
                        v.scalar_tensor_tensor(
                            junkV[:], g_t[:, ms], 1.0, gh[(i - j) % M][:, ms],
                            op0=ALU.bypass, op1=ALU.mult,
                            accum_out=dVm[:, (j - 1) * 4 + m:(j - 1) * 4 + m + 1])

                # ---- partition-reduce dot partials ----
                pball = pps.tile([128, 32], FP, tag="psmall", name="pball")
                psd = pball[0:1, 0:20]
                nc.tensor.matmul(psd[:, 0:4], ones_col[:], dAm[:],
                                 start=True, stop=True)
                nc.tensor.matmul(psd[:, 4:20], ones_col[:], dVm[:],
                                 start=True, stop=True)
                sc.activation(redp[:, 0:20], psd, ACT.Copy)

                do_ar = i >= M
                if do_ar:
                    cc_in = dp.tile([1, 49], FP, tag="cci", name="cci")
                    cc_out = dp.tile([1, 49], FP, tag="cco", name="cco")
                    nc.sync.dma_start(cc_in[0:1, 0:24], redp[:])
                    if i == M:
                        # fuse the early-phase Gram reduction into the same
                        # AllReduce: ship the local P alongside the dots
                        nc.sync.dma_start(cc_in[0:1, 24:49], Pg[(i + 1) % 2][:])
                    gp.collective_compute(
                        "AllReduce", ALU.add, replica_groups=RGROUPS,
                        ins=[cc_in.opt()], outs=[cc_out.opt()],
                    )

                # HAM keep-warm: one long accumulation group of junk matmuls
                # (closed after the solve and READ once, so DCE keeps them)
                # fills the AllReduce wait; solve-keyed members tick the PE
                # through the Gauss-Jordan.
                pdum = None
                if do_ar and NDUM > 0:
                    pdum = pps.tile([1, 512], FP, tag="dum", name="pdum")
                    for k in range(NDUM):
                        nc.tensor.matmul(pdum[:], identR[0:1, 0:1],
                                         xwxp[0:1, 0:512],
                                         start=(k == 0), stop=False)

                if do_ar:
                    nc.sync.dma_start(red2[:], cc_out[0:1, 0:24])
                    # r5[j] = sum over the 4 m-partials of dot j
                    v.tensor_reduce(r5[:],
                                    red2[:, 0:20].rearrange(
                                        "p (j m) -> p j m", j=5),
                                    axis=mybir.AxisListType.X, op=ALU.add)
                else:
                    v.tensor_reduce(r5[:],
                                    redp[:, 0:20].rearrange(
                                        "p (j m) -> p j m", j=5),
                                    axis=mybir.AxisListType.X, op=ALU.add)
                    if i in (2, 4):
                        warm_ar()  # keep the collective path warm

                # ---- P shift + insert (r5 cols: [<g,g>, j1..j4]) ----
                Pc, Pp = Pg[i % 2], Pg[(i + 1) % 2]
                P3c = Pc[:].rearrange("p (a b) -> p a b", a=5)
                P3p = Pp[:].rearrange("p (a b) -> p a b", a=5)
                if i == M:
                    # previous P arrives globally-reduced in the AR payload
                    nc.sync.dma_start(Pp[:], cc_out[0:1, 24:49])
                v.tensor_copy(P3c[:, 1:5, 1:5], P3p[:, 0:4, 0:4])
                v.tensor_copy(Pc[:, 0:5], r5[:, 0:5])
                v.tensor_copy(Pc[:, 5:25:5], r5[:, 1:5])

                if i < M:
                    z_mm = f_t[:]
                    continue

                # ---- build augmented [HTH + lam I | HTy] in Au = [1,4,5] ----
                A3 = Au[:].rearrange("p (a b) -> p a b", a=4)
                v.tensor_tensor(A3[:, :, 0:4],
                                P3c[:, 1:5, 0:1].broadcast_to([1, 4, 4]),
                                P3c[:, 1:5, 1:5], op=ALU.subtract)
                v.scalar_tensor_tensor(u4[:], Pc[:, 1:5], -1.0,
                                       Pc[:, 0:1].broadcast_to([1, 4]),
                                       op0=ALU.mult, op1=ALU.add)
                v.tensor_tensor(A3[:, :, 0:4],
                                u4[:].rearrange("p (a b) -> p a b", a=1)
                                     .broadcast_to([1, 4, 4]),
                                A3[:, :, 0:4], op=ALU.subtract)
                v.tensor_scalar(st4[:], Au[:, 0:19:6], LAM, None, op0=ALU.add)
                v.tensor_copy(Au[:, 0:19:6], st4[:])
                v.scalar_tensor_tensor(
                    A3[:, :, 4:5],
                    P3c[:, 1:5, 0:1], -1.0,
                    Pc[:, 0:1].rearrange("p (a b) -> p a b", a=1)
                              .broadcast_to([1, 4, 1]),
                    op0=ALU.mult, op1=ALU.add)

                # ---- Gauss-Jordan (no pivoting; HTH is SPD + lam I) ----
                # a dummy PE matmul after each pivot keeps the clock hot
                for p in range(4):
                    v.reciprocal(rcp[:], Au[:, p * 6:p * 6 + 1])
                    v.tensor_scalar(rowp[:], Au[:, p * 5:(p + 1) * 5], rcp[:],
                                    None, op0=ALU.mult)
                    v.tensor_tensor(t45[:].rearrange("p (a b) -> p a b", a=4),
                                    A3[:, :, p:p + 1].broadcast_to([1, 4, 5]),
                                    rowp[:].rearrange("p (a b) -> p a b", a=1)
                                           .broadcast_to([1, 4, 5]),
                                    op=ALU.mult)
                    v.tensor_tensor(A3, A3,
                                    t45[:].rearrange("p (a b) -> p a b", a=4),
                                    op=ALU.subtract)
                    v.tensor_copy(Au[:, p * 5:(p + 1) * 5], rowp[:])
                    if pdum is not None:
                        # pace the PE through the solve with all-f32r members
                        v.tensor_copy(pacev[:], rowp[:, 0:4])
                        nc.tensor.matmul(pdum[0:1, 0:4], identR[0:1, 0:1],
                                         pacev[:], start=False, stop=False)

                # gamma = Au[:, 4:20:5]; coeffs = [1 - sum(gamma), gamma]
                v.tensor_reduce(csum[:], Au[:, 4:20:5],
                                axis=mybir.AxisListType.X, op=ALU.add)
                v.tensor_scalar(coeffs[:, 0:1], csum[:], -1.0, 1.0,
                                op0=ALU.mult, op1=ALU.add)
                v.tensor_copy(coeffs[:, 1:5], Au[:, 4:20:5])

                # broadcast coeffs to all partitions, build scaled identities
                psb = pball[:, 20:25]
                nc.tensor.matmul(psb, ones_row[:], coeffs[:],
                                 start=True, stop=True)
                for j in range(M):
                    v.tensor_scalar(identC[j][:], _f32(identR[:]),
                                    psb[:, j:j + 1], None, op0=ALU.mult)

                # close + read the keep-warm group so it survives DCE
                if pdum is not None:
                    v.tensor_copy(pacev[:], coeffs[:, 0:4])
                    nc.tensor.matmul(pdum[0:1, 0:4], identR[0:1, 0:1],
                                     pacev[:], start=False, stop=True)
                    sc.activation(dumout[:], pdum[0:1, 0:4], ACT.Copy)

                # ---- z_{i+1} = sum_k c_k f_{i-k} on the PE ----
                zn = z0 if (i % 2 == 0) else z1
                for kc in range(KD):
                    ks = slice(kc * RPC, (kc + 1) * RPC)
                    psz = ppz.tile([128, RPC], FP, tag="psz", name="psz")
                    for j in range(M):
                        nc.tensor.matmul(psz[:], identC[j][:],
                                         fh[(i - j) % M][:, ks],
                                         start=(j == 0), stop=(j == M - 1))
                    sc.activation(zn[:, ks], psz[:], ACT.Identity)
                z_mm = zn[:]

            for k in range(KD):
                nc.sync.dma_start(zout_d[k * 128:(k + 1) * 128, :],
                                  _f32(z_mm)[:, k * RPC:(k + 1) * RPC])

    nc.compile()
    nc.finalize()
    return nc


_NC = None


def _get_nc():
    global _NC
    if _NC is None:
        nc = bacc.Bacc(trn_type="TRN2", debug=False, num_devices=NCORES)
        _NC = _emit(nc)
    return _NC


def kernel(**inputs):
    x = np.ascontiguousarray(np.asarray(inputs["x_input"], dtype=np.float32))
    W1 = np.ascontiguousarray(np.asarray(inputs["W1"], dtype=np.float32))
    Wx = np.ascontiguousarray(np.asarray(inputs["Wx"], dtype=np.float32))
    b1 = np.ascontiguousarray(np.asarray(inputs["b1"], dtype=np.float32))
    W2 = np.ascontiguousarray(np.asarray(inputs["W2"], dtype=np.float32))
    b2 = np.ascontiguousarray(np.asarray(inputs["b2"], dtype=np.float32))

    nc = _get_nc()
    in_maps = []
    for c in range(NCORES):
        b, s0 = c // 4, (c % 4) * RPC
        in_maps.append({
            "xT": np.ascontiguousarray(x[b, s0:s0 + RPC, :].T),
            "W1": W1, "Wx": Wx, "W2": W2, "b1": b1, "b2": b2,
        })
    res = run_bass_kernel_spmd(nc, in_maps, core_ids=list(range(NCORES)))
    out = np.zeros((B, S, D), np.float32)
    for c, om in enumerate(res.results):
        b, s0 = c // 4, (c % 4) * RPC
        out[b, s0:s0 + RPC, :] = om["zT_out"].T
    return out


# revision 38
# speedup vs baseline: 1.3855x; 1.1632x over previous
"""Trainium2 Bass kernel for the DeepEquilibriumModel (Anderson-accelerated DEQ).

Problem: 12 unrolled iterations of
    f(z) = tanh(z @ W1 + x @ Wx + b1) @ W2 + b2
with Anderson mixing (M=5, beta=1, lam=1e-4) from iteration 5 on.

Sharding: pure data parallelism over the 2048 = B*S rows; 8 cores get 256
rows each (cores 0-3 hold batch 0, cores 4-7 batch 1). Weights replicated.
The Anderson normal equations need global row sums per batch element, done
with a tiny per-group AllReduce (groups {0..3} / {4..7}).

v3 structure (tuned to measured engine rates):
  * h-loop: per f-chunk, ps1 = identity@xwx + sum_k W1[k,f]@z[k]; tanh to a
    full h tile.  W2 phase is m-outer (one PSUM bank at a time) so f, g and
    the Gram dot partials trail each m-chunk instead of waiting for all.
  * dots are m-chunked (16 small DVE ops + 4 ACT squares) -> only ~2.5us of
    tail after the last W2 matmul before the AllReduce can launch.
  * Early iterations (0..3) do NO AllReduce: the Gram matrix P is built from
    local partials and reduced ONCE at i=4 (P is linear in the dots), which
    avoids backing up the collective stream.
  * z_{i+1} = sum_k c_k f_{i-k} runs on the PE as 5 scaled-identity matmuls
    per k-chunk (identC_k built by DVE from the broadcast coefficients),
    with ACT copying PSUM->SBUF; ~3us instead of ~8us of DVE stt chain.
  * 4x4 solve via vectorized Gauss-Jordan (SPD + lam*I, no pivoting) on
    [1,4,5] views; builds are fused (~26 tiny DVE ops total).
  * HAM keep-warm: free-running dummy matmuls fill the AllReduce wait and
    solve-keyed dummies tick the PE through the Gauss-Jordan so the clock
    gate never drops the PE to 1.2 GHz mid-iteration.
"""

import numpy as np

from concourse import bacc, bass, mybir, tile
from concourse.bass_utils import run_bass_kernel_spmd

import os as _os

B, S, D, F = 2, 1024, 512, 2048
MAX_ITER, M, LAM = int(_os.environ.get("K_ITERS", "12")), 5, 1e-4
NCORES = 8
RPC = (B * S) // NCORES      # rows per core = 256
KD = D // 128                # 4 k-chunks over D
KF = F // 128                # 16 k-chunks over F
MD = D // 128                # 4 output chunks over D
NDUM = int(_os.environ.get("K_NDUM", "0"))

FP = mybir.dt.float32
FPR = mybir.dt.float32r
BF = mybir.dt.bfloat16
ALU = mybir.AluOpType
ACT = mybir.ActivationFunctionType

RGROUPS = [[0, 1, 2, 3], [4, 5, 6, 7]]
WT = BF


def _f32(ap):
    return ap.bitcast(FP)


def _emit(nc: bass.Bass):
    v = nc.vector
    sc = nc.scalar
    gp = nc.gpsimd

    # ---------------- DRAM I/O ----------------
    xT_d = nc.dram_tensor("xT", [D, RPC], WT, kind="ExternalInput")
    W1_d = nc.dram_tensor("W1", [D, F], WT, kind="ExternalInput")
    Wx_d = nc.dram_tensor("Wx", [D, F], WT, kind="ExternalInput")
    W2_d = nc.dram_tensor("W2", [F, D], WT, kind="ExternalInput")
    b1_d = nc.dram_tensor("b1", [F], FP, kind="ExternalInput")
    b2_d = nc.dram_tensor("b2", [D], FP, kind="ExternalInput")
    zout_d = nc.dram_tensor("zT_out", [D, RPC], FP, kind="ExternalOutput")

    with tile.TileContext(nc) as tc:
        with (
            tc.tile_pool(name="const", bufs=1) as cp,
            tc.tile_pool(name="ps1p", bufs=2, space="PSUM") as pp1,
            tc.tile_pool(name="ps2p", bufs=2, space="PSUM") as pp2,
            tc.tile_pool(name="pszp", bufs=2, space="PSUM") as ppz,
            tc.tile_pool(name="pssm", bufs=1, space="PSUM") as pps,
            tc.tile_pool(name="dram", bufs=2, space="DRAM") as dp,
        ):
            # ---------------- constants / weights ----------------
            W1p = cp.tile([128, KD * F], WT)          # (k,f) at [:, k*F + f*128]
            W2p = cp.tile([128, KF * D], WT)          # (f,m) at [:, f*D + m*128]
            xwxp = cp.tile([128, KF * RPC], WT)       # f at [:, f*RPC]
            b1t = cp.tile([128, KF], FP)
            b2t = cp.tile([128, MD], FP)
            ones_col = cp.tile([128, 1], FP)
            ones_row = cp.tile([1, 128], FP)
            onesq = cp.tile([128, 128], FP)
            identR = cp.tile([128, 128], WT)
            identC = [cp.tile([128, 128], WT, name=f"idc{j}") for j in range(M)]

            nc.sync.dma_start(b1t[:], b1_d.ap().rearrange("(f p) -> p f", p=128))
            nc.sync.dma_start(b2t[:], b2_d.ap().rearrange("(m p) -> p m", p=128))
            v.memset(ones_col[:], 1.0)
            v.memset(ones_row[:], 1.0)
            v.memset(onesq[:], 1.0)
            gp.affine_select(onesq[:], onesq[:], [[1, 128]], ALU.is_equal, 0.0,
                             base=0, channel_multiplier=-1)
            v.tensor_copy(identR[:], onesq[:])

            # -------- state tiles --------
            fh = [cp.tile([128, KD * RPC], WT, name=f"fh{j}") for j in range(M)]
            gh = [cp.tile([128, KD * RPC], BF, name=f"gh{j}") for j in range(M)]
            z0 = cp.tile([128, KD * RPC], WT)
            z1 = cp.tile([128, KD * RPC], WT)
            z320 = cp.tile([128, KD * RPC], FP)
            z321 = cp.tile([128, KD * RPC], FP)
            hfull = cp.tile([128, KF * RPC], WT)
            junkV = cp.tile([128, RPC], BF)
            junkA = cp.tile([128, RPC], BF)
            dAm = cp.tile([128, 4], FP)               # <g,g> partials by m
            dVm = cp.tile([128, 16], FP)              # (j,m) partials, j-major
            redp = cp.tile([1, 24], FP)
            red2 = cp.tile([1, 24], FP)
            r5 = cp.tile([1, 5], FP)
            Pg = [cp.tile([1, 25], FP, name=f"pg{j}") for j in range(2)]
            Au = cp.tile([1, 20], FP)      # augmented [HTH | HTy] as [1,4,5]
            u4 = cp.tile([1, 4], FP)
            st4 = cp.tile([1, 4], FP)
            rcp = cp.tile([1, 1], FP)
            rowp = cp.tile([1, 5], FP)
            t45 = cp.tile([1, 20], FP)
            csum = cp.tile([1, 1], FP)
            coeffs = cp.tile([1, 5], FP)
            dumout = cp.tile([1, 4], FP)
            pacev = cp.tile([1, 4], WT)

            v.memset(dAm[:], 0.0)
            v.memset(dVm[:], 0.0)
            v.memset(redp[:], 0.0)
            v.memset(Pg[0][:], 0.0)
            v.memset(Pg[1][:], 0.0)

            # warm up the collective path (first AllReduce after load pays a
            # large one-time latency).
            n_warm = int(_os.environ.get("K_CC_WARMUP", "2"))

            def warm_ar():
                wcc_in = dp.tile([1, 49], FP, tag="cci", name="wcci")
                wcc_out = dp.tile([1, 49], FP, tag="cco", name="wcco")
                gp.dma_start(wcc_in[0:1, 0:24], redp[:])
                gp.collective_compute(
                    "AllReduce", ALU.add, replica_groups=RGROUPS,
                    ins=[wcc_in.opt()], outs=[wcc_out.opt()],
                )

            for w in range(n_warm):
                warm_ar()

            # ---------------- prolog: xwx = Wx.T @ xT + b1 ----------------
            with tc.tile_pool(name="prolog", bufs=1) as pro:
                xTs = pro.tile([128, KD * RPC], WT)
                Wxp = pro.tile([128, KD * F], WT)
                # two hardware DMA queues in parallel: Wx on the scalar
                # engine's queue, everything else on sync
                for k in range(KD):
                    sc.dma_start(Wxp[:, k * F:(k + 1) * F],
                                 Wx_d[k * 128:(k + 1) * 128, :])
                for k in range(KD):
                    nc.sync.dma_start(xTs[:, k * RPC:(k + 1) * RPC],
                                      xT_d[k * 128:(k + 1) * 128, :])
                for f in range(KF):
                    nc.sync.dma_start(W2p[:, f * D:(f + 1) * D],
                                      W2_d[f * 128:(f + 1) * 128, :])
                for k in range(KD):
                    nc.sync.dma_start(W1p[:, k * F:(k + 1) * F],
                                      W1_d[k * 128:(k + 1) * 128, :])
                for f in range(KF):
                    ps1 = pp1.tile([128, RPC], FP, tag="ps1", name="ps1")
                    for k in range(KD):
                        nc.tensor.matmul(
                            ps1[:],
                            Wxp[:, k * F + f * 128: k * F + (f + 1) * 128],
                            xTs[:, k * RPC:(k + 1) * RPC],
                            start=(k == 0), stop=(k == KD - 1),
                        )
                    sc.activation(xwxp[:, f * RPC:(f + 1) * RPC], ps1[:],
                                  ACT.Identity, bias=b1t[:, f:f + 1], scale=1.0)

            # ---------------- main loop (fully unrolled) ----------------
            z_mm = None   # bf16 AP of z_i for matmuls (None for i=0 -> zeros)
            z_sub = None  # AP used by the g subtraction (fp32 from i=6 on)
            for i in range(MAX_ITER):
                slot = i % M
                last = (i == MAX_ITER - 1)
                f_t, g_t = fh[slot], gh[slot]

                # ---- h phase: hfull = tanh(z @ W1 + xwx) ----
                for f in range(KF):
                    fs = slice(f * RPC, (f + 1) * RPC)
                    if i == 0:
                        sc.activation(hfull[:, fs], xwxp[:, fs], ACT.Tanh)
                        continue
                    ps1 = pp1.tile([128, RPC], FP, tag="ps1", name="ps1")
                    nc.tensor.matmul(ps1[:], identR[:], xwxp[:, fs],
                                     start=True, stop=False)
                    for k in range(KD):
                        nc.tensor.matmul(
                            ps1[:],
                            W1p[:, k * F + f * 128: k * F + (f + 1) * 128],
                            z_mm[:, k * RPC:(k + 1) * RPC],
                            start=False, stop=(k == KD - 1),
                        )
                    sc.activation(hfull[:, fs], ps1[:], ACT.Tanh)

                # ---- W2 phase, m-outer; f/g/dot partials trail each m ----
                njd = min(i, M - 1)
                for m in range(MD):
                    ms = slice(m * RPC, (m + 1) * RPC)
                    ps2 = pp2.tile([128, RPC], FP, tag="ps2", name="ps2")
                    for f in range(KF):
                        nc.tensor.matmul(
                            ps2[:],
                            W2p[:, f * D + m * 128: f * D + (m + 1) * 128],
                            hfull[:, f * RPC:(f + 1) * RPC],
                            start=(f == 0), stop=(f == KF - 1),
                        )
                    sc.activation(f_t[:, ms], ps2[:],
                                  ACT.Identity, bias=b2t[:, m:m + 1], scale=1.0)
                    if i == 0:
                        v.tensor_scalar(g_t[:, ms], ps2[:], b2t[:, m:m + 1],
                                        None, op0=ALU.add)
                    else:
                        v.scalar_tensor_tensor(g_t[:, ms], ps2[:],
                                               b2t[:, m:m + 1], z_sub[:, ms],
                                               op0=ALU.add, op1=ALU.subtract)
                    sc.activation(junkA[:], g_t[:, ms], ACT.Square,
                                  accum_out=dAm[:, m:m + 1])
                    for j in range(1, njd + 1):
                        v.scalar_tensor_tensor(
                            junkV[:], g_t[:, ms], 1.0, gh[(i - j) % M][:, ms],
                            op0=ALU.bypass, op1=ALU.mult,
                            accum_out=dVm[:, (j - 1) * 4 + m:(j - 1) * 4 + m + 1])

                # ---- partition-reduce dot partials ----
                pball = pps.tile([128, 32], FP, tag="psmall", name="pball")
                psd = pball[0:1, 0:20]
                nc.tensor.matmul(psd[:, 0:4], ones_col[:], dAm[:],
                                 start=True, stop=True)
                nc.tensor.matmul(psd[:, 4:20], ones_col[:], dVm[:],
                                 start=True, stop=True)
                sc.activation(redp[:, 0:20], psd, ACT.Copy)

                do_ar = i >= M
                if do_ar:
                    cc_in = dp.tile([1, 49], FP, tag="cci", name="cci")
                    cc_out = dp.tile([1, 49], FP, tag="cco", name="cco")
                    nc.sync.dma_start(cc_in[0:1, 0:24], redp[:])
                    if i == M:
                        # fuse the early-phase Gram reduction into the same
                        # AllReduce: ship the local P alongside the dots
                        nc.sync.dma_start(cc_in[0:1, 24:49], Pg[(i + 1) % 2][:])
                    gp.collective_compute(
                        "AllReduce", ALU.add, replica_groups=RGROUPS,
                        ins=[cc_in.opt()], outs=[cc_out.opt()],
                    )

                # HAM keep-warm: one long accumulation group of junk matmuls
                # (closed after the solve and READ once, so DCE keeps them)
                # fills the AllReduce wait; solve-keyed members tick the PE
                # through the Gauss-Jordan.
                pdum = None
                if do_ar and NDUM > 0:
                    pdum = pps.tile([1, 512], FP, tag="dum", name="pdum")
                    for k in range(NDUM):
                        nc.tensor.matmul(pdum[:], identR[0:1, 0:1],
                                         xwxp[0:1, 0:512],
                                         start=(k == 0), stop=False)

                if do_ar:
                    nc.sync.dma_start(red2[:], cc_out[0:1, 0:24])
                    # r5[j] = sum over the 4 m-partials of dot j
                    v.tensor_reduce(r5[:],
                                    red2[:, 0:20].rearrange(
                                        "p (j m) -> p j m", j=5),
                                    axis=mybir.AxisListType.X, op=ALU.add)
                else:
                    v.tensor_reduce(r5[:],
                                    redp[:, 0:20].rearrange(
                                        "p (j m) -> p j m", j=5),
                                    axis=mybir.AxisListType.X, op=ALU.add)
                    if i in (2, 4):
                        warm_ar()  # keep the collective path warm

                # ---- P shift + insert (r5 cols: [<g,g>, j1..j4]) ----
                Pc, Pp = Pg[i % 2], Pg[(i + 1) % 2]
                P3c = Pc[:].rearrange("p (a b) -> p a b", a=5)
                P3p = Pp[:].rearrange("p (a b) -> p a b", a=5)
                if i == M:
                    # previous P arrives globally-reduced in the AR payload
                    nc.sync.dma_start(Pp[:], cc_out[0:1, 24:49])
                v.tensor_copy(P3c[:, 1:5, 1:5], P3p[:, 0:4, 0:4])
                v.tensor_copy(Pc[:, 0:5], r5[:, 0:5])
                v.tensor_copy(Pc[:, 5:25:5], r5[:, 1:5])

                if i < M:
                    z_mm = f_t[:]
                    z_sub = f_t[:]
                    continue

                # ---- build augmented [HTH + lam I | HTy] in Au = [1,4,5] ----
                A3 = Au[:].rearrange("p (a b) -> p a b", a=4)
                v.tensor_tensor(A3[:, :, 0:4],
                                P3c[:, 1:5, 0:1].broadcast_to([1, 4, 4]),
                                P3c[:, 1:5, 1:5], op=ALU.subtract)
                v.scalar_tensor_tensor(u4[:], Pc[:, 1:5], -1.0,
                                       Pc[:, 0:1].broadcast_to([1, 4]),
                                       op0=ALU.mult, op1=ALU.add)
                v.tensor_tensor(A3[:, :, 0:4],
                                u4[:].rearrange("p (a b) -> p a b", a=1)
                                     .broadcast_to([1, 4, 4]),
                                A3[:, :, 0:4], op=ALU.subtract)
                v.tensor_scalar(st4[:], Au[:, 0:19:6], LAM, None, op0=ALU.add)
                v.tensor_copy(Au[:, 0:19:6], st4[:])
                v.scalar_tensor_tensor(
                    A3[:, :, 4:5],
                    P3c[:, 1:5, 0:1], -1.0,
                    Pc[:, 0:1].rearrange("p (a b) -> p a b", a=1)
                              .broadcast_to([1, 4, 1]),
                    op0=ALU.mult, op1=ALU.add)

                # ---- Gauss-Jordan (no pivoting; HTH is SPD + lam I) ----
                # a dummy PE matmul after each pivot keeps the clock hot
                for p in range(4):
                    v.reciprocal(rcp[:], Au[:, p * 6:p * 6 + 1])
                    v.tensor_scalar(rowp[:], Au[:, p * 5:(p + 1) * 5], rcp[:],
                                    None, op0=ALU.mult)
                    v.tensor_tensor(t45[:].rearrange("p (a b) -> p a b", a=4),
                                    A3[:, :, p:p + 1].broadcast_to([1, 4, 5]),
                                    rowp[:].rearrange("p (a b) -> p a b", a=1)
                                           .broadcast_to([1, 4, 5]),
                                    op=ALU.mult)
                    v.tensor_tensor(A3, A3,
                                    t45[:].rearrange("p (a b) -> p a b", a=4),
                                    op=ALU.subtract)
                    v.tensor_copy(Au[:, p * 5:(p + 1) * 5], rowp[:])
                    if pdum is not None:
                        # pace the PE through the solve with all-f32r members
                        v.tensor_copy(pacev[:], rowp[:, 0:4])
                        nc.tensor.matmul(pdum[0:1, 0:4], identR[0:1, 0:1],
                                         pacev[:], start=False, stop=False)

                # gamma = Au[:, 4:20:5]; coeffs = [1 - sum(gamma), gamma]
                v.tensor_reduce(csum[:], Au[:, 4:20:5],
                                axis=mybir.AxisListType.X, op=ALU.add)
                v.tensor_scalar(coeffs[:, 0:1], csum[:], -1.0, 1.0,
                                op0=ALU.mult, op1=ALU.add)
                v.tensor_copy(coeffs[:, 1:5], Au[:, 4:20:5])

                # broadcast coeffs to all partitions, build scaled identities
                psb = pball[:, 20:25]
                nc.tensor.matmul(psb, ones_row[:], coeffs[:],
                                 start=True, stop=True)
                for j in range(M):
                    v.tensor_scalar(identC[j][:], identR[:],
                                    psb[:, j:j + 1], None, op0=ALU.mult)

                # close + read the keep-warm group so it survives DCE
                if pdum is not None:
                    v.tensor_copy(pacev[:], coeffs[:, 0:4])
                    nc.tensor.matmul(pdum[0:1, 0:4], identR[0:1, 0:1],
                                     pacev[:], start=False, stop=True)
                    sc.activation(dumout[:], pdum[0:1, 0:4], ACT.Copy)

                # ---- z_{i+1} = sum_k c_k f_{i-k} on the PE ----
                zn = z0 if (i % 2 == 0) else z1
                zn32 = z320 if (i % 2 == 0) else z321
                for kc in range(KD):
                    ks = slice(kc * RPC, (kc + 1) * RPC)
                    psz = ppz.tile([128, RPC], FP, tag="psz", name="psz")
                    for j in range(M):
                        nc.tensor.matmul(psz[:], identC[j][:],
                                         fh[(i - j) % M][:, ks],
                                         start=(j == 0), stop=(j == M - 1))
                    sc.activation(zn[:, ks], psz[:], ACT.Identity)
                    v.tensor_copy(zn32[:, ks], psz[:])
                z_mm = zn[:]
                z_sub = zn32[:]

            for k in range(KD):
                nc.sync.dma_start(zout_d[k * 128:(k + 1) * 128, :],
                                  z_sub[:, k * RPC:(k + 1) * RPC])

    nc.compile()
    nc.finalize()
    return nc


_NC = None


def _get_nc():
    global _NC
    if _NC is None:
        nc = bacc.Bacc(trn_type="TRN2", debug=False, num_devices=NCORES)
        _NC = _emit(nc)
    return _NC


def kernel(**inputs):
    import ml_dtypes
    bf = ml_dtypes.bfloat16
    x = np.ascontiguousarray(np.asarray(inputs["x_input"], dtype=np.float32))
    W1 = np.ascontiguousarray(np.asarray(inputs["W1"], dtype=np.float32).astype(bf))
    Wx = np.ascontiguousarray(np.asarray(inputs["Wx"], dtype=np.float32).astype(bf))
    b1 = np.ascontiguousarray(np.asarray(inputs["b1"], dtype=np.float32))
    W2 = np.ascontiguousarray(np.asarray(inputs["W2"], dtype=np.float32).astype(bf))
    b2 = np.ascontiguousarray(np.asarray(inputs["b2"], dtype=np.float32))

    nc = _get_nc()
    in_maps = []
    for c in range(NCORES):
        b, s0 = c // 4, (c % 4) * RPC
        in_maps.append({
            "xT": np.ascontiguousarray(x[b, s0:s0 + RPC, :].T.astype(bf)),
            "W1": W1, "Wx": Wx, "W2": W2, "b1": b1, "b2": b2,
        })
    res = run_bass_kernel_spmd(nc, in_maps, core_ids=list(range(NCORES)))
    out = np.zeros((B, S, D), np.float32)
    for c, om in enumerate(res.results):
        b, s0 = c // 4, (c % 4) * RPC
        out[b, s0:s0 + RPC, :] = om["zT_out"].T
    return out
